# revision 4
# baseline (speedup 1.0000x reference)
"""Trainium2 Bass kernel for nn_EquivariantProductBasisBlock (MACE symmetric
contraction, correlation 3, irreps 0e+1o -> 0e+1o, + e3nn linear).

Strategy (data-parallel over nodes, 8 cores):
  Per core: 64 nodes x 64 channels = 4096 (b,c) pairs, each with a 9-dim
  feature vector x.  The full contraction reduces to, per pair:
      T[(D,q)] = sum_f  F[f] * Ucat[f, (D,q)]          (matmul, f = 219)
      f[D]     = sum_q  Wexp[(D,q)] * T[(D,q)]          (species weights)
      out      = blockdiag(Wlin) applied over channels  (matmul)
  where F = [x (9) | sym pairs x_j x_k (45) | sym triples x_i x_j x_k (165)]
  and Ucat folds the (symmetric) U3/U2/U1 CG tensors with permutation
  multiplicities.  Species gather + weight packing happen host-side.

Engine schedule (v2):
  DMA:   x first, then cblob/wblob; f1-chunk transposes via DMA xbar
         (SBUF->SBUF, 16x128 tiles) straight into ft1 -- no PSUM, no evac.
  PE:    warmup (HAM ramp), f0-chunk transposes (is_transpose bf16 PSUM),
         T matmuls (K=128 chunk0 + K=91 chunk1 accumulate), final linear.
  DVE:   monomial pairs + big triple segs, f0-evac (2x packed), q-reduce.
  Pool:  small triple segs, W-mult (SBUF bf16).
  Act:   T-evac (fp32 PSUM -> bf16 SBUF), output copy.
"""

import os
import sys

for _p in ("/opt/trn_rl_repo",):
    if _p not in sys.path:
        sys.path.insert(0, _p)

import numpy as np
import ml_dtypes

N_CORES = 8
N_NODES = 512
B = N_NODES // N_CORES  # nodes per core
C = 64                  # channels
NF = 9                  # features per channel
BC = B * C              # 4096 pairs per core
G = BC // 128           # 32 partition tiles
K3, K2, K1 = 16, 4, 1
NQ = K3 + K2 + K1       # 21
ND = 4                  # output dims: idx0 d=1, idx1 d=3
MUL = 64

# Symmetric bases ------------------------------------------------------------
PAIRS = [(j, k) for j in range(NF) for k in range(j, NF)]  # 45, j<=k
TRI2 = {jk: t for t, jk in enumerate(PAIRS)}
NP2 = len(PAIRS)  # 45
SEG_OFF = []
SEG_LEN = []
_off = 0
for i in range(NF):
    SEG_OFF.append(_off)
    SEG_LEN.append(NP2 - TRI2[(i, i)])
    _off += SEG_LEN[-1]
NP3 = _off  # 165
NFEAT_TOT = NF + NP2 + NP3  # 219
CH0_N = 128
CH1_N = NFEAT_TOT - CH0_N   # 91

F_COL_X = 0
F_COL_P2 = NF          # 9
F_COL_P3 = NF + NP2    # 54

BF16 = ml_dtypes.bfloat16

N_WARM = int(os.environ.get("K_WARM", "10"))
# triple segments computed on gpsimd (rest on DVE)
GPSEGS = set(int(s) for s in os.environ.get("K_GPSEGS", "4,5,6,7,8").split(",")
             if s != "")
# W-mult engine: "pool" (Act evacs T to SBUF first) or "dve" (direct PSUM)
WMULT = os.environ.get("K_WMULT", "pool")
# f1 transpose path: "xbar" (DMA) or "pe"
F1T = os.environ.get("K_F1T", "xbar")
TB = 4   # g-tiles per T-psum batch (one PSUM bank)
EB = 8   # g-tiles per f0-transpose psum bank (bf16)

_CACHE = {}


def _mult3(i, j, k):
    if i == j == k:
        return 1.0
    if i == j or j == k or i == k:
        return 3.0
    return 6.0


def _host_pack(node_feats, node_specie,
               U3_0, U2_0, U1_0, w3_0, w2_0, w1_0,
               U3_1, U2_1, U1_1, w3_1, w2_1, w1_1,
               Wlin0, Wlin1):
    node_feats = np.asarray(node_feats, np.float32)
    spec = np.asarray(node_specie).astype(np.int64)

    # --- Ucat [219, 84] ---
    ucat = np.zeros((NFEAT_TOT, ND * NQ), np.float32)
    Us = [(np.asarray(U3_0, np.float32), np.asarray(U2_0, np.float32),
           np.asarray(U1_0, np.float32)),
          (np.asarray(U3_1, np.float32), np.asarray(U2_1, np.float32),
           np.asarray(U1_1, np.float32))]
    for D in range(ND):
        idx = 0 if D == 0 else 1
        d = 0 if D == 0 else D - 1
        U3, U2, U1 = Us[idx]
        col = D * NQ
        ucat[F_COL_X:F_COL_X + NF, col + K3 + K2] = U1[d, :, 0]
        for t, (j, k) in enumerate(PAIRS):
            m2 = 1.0 if j == k else 2.0
            ucat[F_COL_P2 + t, col + K3:col + K3 + K2] = m2 * U2[d, j, k, :]
        for i in range(NF):
            for s, (j, k) in enumerate(PAIRS[TRI2[(i, i)]:]):
                r = F_COL_P3 + SEG_OFF[i] + s
                ucat[r, col:col + K3] = _mult3(i, j, k) * U3[d, i, j, k, :]
    u0 = ucat[0:CH0_N].copy()
    u1 = ucat[CH0_N:NFEAT_TOT].copy()

    # --- per-node species weights ---
    wcat = np.concatenate([
        np.asarray(w3_0, np.float32), np.asarray(w2_0, np.float32),
        np.asarray(w1_0, np.float32), np.asarray(w3_1, np.float32),
        np.asarray(w2_1, np.float32), np.asarray(w1_1, np.float32),
    ], axis=1)                      # [NSPEC, 42, C]
    wnode = wcat[spec]              # [512, 42, C]

    # --- block-diag Wlin [2, 128, 128] (path norm 1/sqrt(C) folded in) ---
    inv_sqrt_c = 1.0 / np.sqrt(np.float32(C))
    bw = np.zeros((2, 128, 128), np.float32)
    for b2 in range(2):
        bw[0, b2 * 64:(b2 + 1) * 64, b2 * 64:(b2 + 1) * 64] = \
            np.asarray(Wlin0, np.float32) * inv_sqrt_c
        bw[1, b2 * 64:(b2 + 1) * 64, b2 * 64:(b2 + 1) * 64] = \
            np.asarray(Wlin1, np.float32) * inv_sqrt_c

    ident = np.eye(128, dtype=np.float32)

    # one [128, 552] bf16 blob: u0 | u1 | bw0 | bw1 | ident
    cblob = np.zeros((128, 552), np.float32)
    cblob[:, 0:84] = u0
    cblob[0:CH1_N, 84:168] = u1
    cblob[:, 168:296] = bw[0]
    cblob[:, 296:424] = bw[1]
    cblob[:, 424:552] = ident
    cblob = cblob.astype(BF16)

    in_maps = []
    for core in range(N_CORES):
        b0 = core * B
        # x pre-shuffled to device layout [128=(b2,c), i, g], bf16
        xs = node_feats[b0:b0 + B].reshape(G, 2, C, NF)      # [g, b2, c, i]
        xs = np.ascontiguousarray(xs.transpose(1, 2, 3, 0))  # [b2, c, i, g]
        xs = xs.reshape(128, NF, G).astype(BF16)
        wex42 = wnode[b0:b0 + B]                             # [B, 42, C]
        # natural layout [128=(b2,c), g, (D,q)=84]
        wex84 = np.concatenate(
            [wex42[:, 0:NQ]] + [wex42[:, NQ:2 * NQ]] * 3, axis=1)  # [B,84,C]
        wn = wex84.reshape(G, 2, ND * NQ, C)                 # [g, b2, 84, c]
        wn = np.ascontiguousarray(wn.transpose(1, 3, 0, 2))  # [b2, c, g, 84]
        wblob = wn.reshape(128, G, ND * NQ)
        in_maps.append({
            "x": xs,
            "cblob": cblob,
            "wblob": wblob.astype(BF16),
        })
    return in_maps


def _host_unpack(res):
    """Device returns o [128=(b2,M), 128] per core; reassemble [512, 256]."""
    out = np.zeros((N_NODES, ND * MUL), np.float32)
    for core in range(N_CORES):
        o = res[core]["o"]                       # [128, 128]
        o = o.reshape(2, MUL, 128)               # [b2, M, col]
        b0 = core * B
        # col 0..31 = g (D0);  col 32.. = (g, i)
        o0 = o[:, :, 0:G]                        # [b2, M, g]
        o1 = o[:, :, G:G + 3 * G].reshape(2, MUL, G, 3)
        for b2 in range(2):
            rows = b0 + 2 * np.arange(G) + b2    # [g]
            out[rows, 0:MUL] = o0[b2].T          # [g, M]
            cols = (MUL + 3 * np.arange(MUL)[None, :, None]
                    + np.arange(3)[None, None, :])      # [1, M, 3]
            out[rows[:, None, None], cols] = o1[b2].transpose(1, 0, 2)
    return out


def _build_nc():
    import concourse.bass as bass
    import concourse.tile as tile
    from concourse import mybir, bacc

    F32 = mybir.dt.float32
    BF = mybir.dt.bfloat16

    nc = bacc.Bacc("TRN2", target_bir_lowering=False, debug=False,
                   num_devices=N_CORES)

    x_d = nc.dram_tensor("x", [128, NF, G], BF, kind="ExternalInput").ap()
    cblob_d = nc.dram_tensor("cblob", [128, 552], BF,
                             kind="ExternalInput").ap()
    wblob_d = nc.dram_tensor("wblob", [128, G, ND * NQ], BF,
                             kind="ExternalInput").ap()
    o_d = nc.dram_tensor("o", [128, 128], F32, kind="ExternalOutput").ap()

    with tile.TileContext(nc) as tc:
        with (
            tc.tile_pool(name="sb", bufs=1) as sbp,
            tc.tile_pool(name="ps", bufs=1, space="PSUM") as psp,
        ):
            # ---- tiles (allocated once; manual rotation) ----
            f0 = sbp.tile([128, 128, G], BF)       # g-innermost (2x DVE)
            f1 = sbp.tile([128, G, 128], BF)       # g-major (xbar-friendly);
            #                                        f-cols 91.. garbage
            cb_sb = sbp.tile([128, 552], BF)
            wb_sb = sbp.tile([128, G, ND * NQ], BF)
            ft0 = sbp.tile([128, BC], BF)
            ft1 = sbp.tile([128, BC], BF)      # rows 91.. garbage
            tsb = [sbp.tile([128, TB, ND * NQ], BF, name=f"tsb{i}")
                   for i in range(2)]
            gsc = [sbp.tile([128, TB, ND * NQ], BF, name=f"gsc{i}")
                   for i in range(2)]
            f_sb = sbp.tile([128, G, ND], BF)
            o_sb = sbp.tile([128, 128], F32)

            warm_ps = psp.tile([128, 512], F32, name="warm")
            o_ps = psp.tile([128, 128], F32, name="ops")
            tp0 = [psp.tile([128, EB, 128], BF, name=f"tp0{i}")
                   for i in range(2)]
            t_ps = [psp.tile([128, TB, ND * NQ], F32, name=f"tps{i}")
                    for i in range(2)]

            u0_sb = cb_sb[:, 0:84]
            u1_sb = cb_sb[0:CH1_N, 84:168]
            bw0_sb = cb_sb[:, 168:296]
            bw1_sb = cb_sb[:, 296:424]
            id_sb = cb_sb[:, 424:552]

            # ---- inputs: x FIRST (it gates the whole pipeline) ----
            nc.sync.dma_start(f0[:, 0:NF, :], x_d[:, 0:NF])
            nc.scalar.dma_start(cb_sb[:], cblob_d)
            half = G // 2
            nc.sync.dma_start(wb_sb[:, 0:half], wblob_d[:, 0:half])
            nc.scalar.dma_start(wb_sb[:, half:G], wblob_d[:, half:G])

            # PE warmup gated on x: flips the HAM window before the real
            # matmuls start.
            if N_WARM:
                wrhs = f0[:, 0:NF, :]
                for w in range(N_WARM):
                    nc.tensor.matmul(warm_ps[:, 0:G * NF], id_sb, wrhs,
                                     start=True, stop=True)

            # ---- monomials ----
            # pairs: rows 9..53 of F (DVE)
            for j in range(NF):
                n = NF - j
                t0 = TRI2[(j, j)]
                nc.vector.tensor_mul(
                    f0[:, F_COL_P2 + t0:F_COL_P2 + t0 + n, :],
                    f0[:, j:j + 1, :].broadcast_to([128, n, G]),
                    f0[:, j:NF, :])

            # triples: seg i = x_i * pairs[t0(i,i):], split across f0/f1.
            # f0 parts run 2x-packed (g innermost); f1 parts are g-major
            # (1x) so the DMA xbar can transpose them contiguously.
            for i in range(NF):
                t0 = TRI2[(i, i)]
                o = F_COL_P3 + SEG_OFF[i]
                n = SEG_LEN[i]
                eng = nc.gpsimd if i in GPSEGS else nc.vector
                if o < 128:
                    w = min(n, 128 - o)
                    toff = F_COL_P2 + t0
                    eng.tensor_mul(
                        f0[:, o:o + w, :],
                        f0[:, i:i + 1, :].broadcast_to([128, w, G]),
                        f0[:, toff:toff + w, :])
                    o += w
                    n -= w
                if n > 0:
                    c0 = o - 128
                    toff = F_COL_P2 + t0 + (o - (F_COL_P3 + SEG_OFF[i]))
                    xi = f0[:, i:i + 1, :].rearrange(
                        "p w g -> p g w").broadcast_to([128, G, n])
                    pr = f0[:, toff:toff + n, :].rearrange("p w g -> p g w")
                    eng.tensor_mul(f1[:, :, c0:c0 + n], xi, pr)

            # ---- f1 transposes via DMA xbar (SBUF->SBUF), or PE ----
            if F1T == "xbar":
                for g in range(G):
                    eng = nc.sync if g % 2 == 0 else nc.scalar
                    eng.dma_start(ft1[:, g * 128:(g + 1) * 128],
                                  f1[:, g, :], transpose=True)

            # ---- f0 transposes (PE, bf16 PSUM) + DVE evac ----
            for k in range(G // EB):
                p0 = tp0[k % 2]
                for e in range(EB):
                    g = k * EB + e
                    nc.tensor.transpose(p0[:, e], f0[:, :, g], id_sb)
                cols = slice(k * EB * 128, (k + 1) * EB * 128)
                nc.vector.tensor_scalar_mul(ft0[:, cols], p0[:], 1.0)
                if F1T == "pe":
                    p1 = tp0[(k + 1) % 2]
                    for e in range(EB):
                        g = k * EB + e
                        nc.tensor.transpose(p1[:, e], f1[:, g, :], id_sb)
                    nc.scalar.copy(ft1[:, cols], p1[:])

            # ---- T matmuls + species weights + q-reduce ----
            for nb in range(G // TB):
                tp = t_ps[nb % 2]
                for e in range(TB):
                    g = nb * TB + e
                    cols = slice(g * 128, (g + 1) * 128)
                    nc.tensor.matmul(tp[:, e], ft0[:, cols], u0_sb,
                                     start=True, stop=False)
                    nc.tensor.matmul(tp[:, e], ft1[0:CH1_N, cols], u1_sb,
                                     start=False, stop=True)
                gs = slice(nb * TB, (nb + 1) * TB)
                if WMULT == "pool":
                    ts = tsb[nb % 2]
                    nc.scalar.copy(ts[:], tp[:])
                    nc.gpsimd.tensor_mul(gsc[nb % 2][:], wb_sb[:, gs], ts[:])
                else:
                    nc.vector.tensor_mul(gsc[nb % 2][:], wb_sb[:, gs], tp[:])
                with nc.allow_low_precision(
                        reason="DVE reduce accumulates fp32 internally"):
                    nc.vector.tensor_reduce(
                        f_sb[:, gs], gsc[nb % 2][:].rearrange(
                            "p g (d q) -> p g d q", q=NQ),
                        axis=mybir.AxisListType.X, op=mybir.AluOpType.add)

            # ---- final linear (block-diag Wlin over channels) ----
            nc.tensor.matmul(o_ps[:, 0:G], bw0_sb, f_sb[:, :, 0],
                             start=True, stop=True)
            nc.tensor.matmul(
                o_ps[:, G:G + G * 3].rearrange("p (g i) -> p g i", g=G),
                bw1_sb, f_sb[:, :, 1:4], start=True, stop=True)

            # ---- output ----
            nc.scalar.copy(o_sb[:], o_ps[:])
            nc.sync.dma_start(o_d, o_sb[:])

    nc.compile()
    return nc


def _get_nc():
    if "nc" not in _CACHE:
        _CACHE["nc"] = _build_nc()
    return _CACHE["nc"]


def kernel(node_feats, node_specie,
           U3_0, U2_0, U1_0, w3_0, w2_0, w1_0,
           U3_1, U2_1, U1_1, w3_1, w2_1, w1_1,
           Wlin0, Wlin1):
    from concourse.bass_utils import run_bass_kernel_spmd

    in_maps = _host_pack(node_feats, node_specie,
                         U3_0, U2_0, U1_0, w3_0, w2_0, w1_0,
                         U3_1, U2_1, U1_1, w3_1, w2_1, w1_1,
                         Wlin0, Wlin1)
    nc = _get_nc()
    res = run_bass_kernel_spmd(nc, in_maps, core_ids=list(range(N_CORES)))
    return _host_unpack(res.results).astype(np.float32)


# revision 6
# speedup vs baseline: 1.4301x; 1.4301x over previous
"""Trainium2 Bass kernel for nn_EquivariantProductBasisBlock (MACE symmetric
contraction, correlation 3, irreps 0e+1o -> 0e+1o, + e3nn linear).

Strategy (data-parallel over nodes, 8 cores):
  Per core: 64 nodes x 64 channels = 4096 (b,c) pairs, each with a 9-dim
  feature vector x.  The full contraction reduces to, per pair:
      T[(D,q)] = sum_f  F[f] * Ucat[f, (D,q)]          (matmul, f = 219)
      f[D]     = sum_q  Wexp[(D,q)] * T[(D,q)]          (species weights)
      out      = blockdiag(Wlin) applied over channels  (matmul)
  where F = [x (9) | sym pairs x_j x_k (45) | sym triples x_i x_j x_k (165)]
  and Ucat folds the (symmetric) U3/U2/U1 CG tensors with permutation
  multiplicities.  Species gather + weight packing happen host-side.

Engine schedule (v2):
  DMA:   x first, then cblob/wblob; f1-chunk transposes via DMA xbar
         (SBUF->SBUF, 16x128 tiles) straight into ft1 -- no PSUM, no evac.
  PE:    warmup (HAM ramp), f0-chunk transposes (is_transpose bf16 PSUM),
         T matmuls (K=128 chunk0 + K=91 chunk1 accumulate), final linear.
  DVE:   monomial pairs + big triple segs, f0-evac (2x packed), q-reduce.
  Pool:  small triple segs, W-mult (SBUF bf16).
  Act:   T-evac (fp32 PSUM -> bf16 SBUF), output copy.
"""

import os
import sys

for _p in ("/opt/trn_rl_repo",):
    if _p not in sys.path:
        sys.path.insert(0, _p)

import numpy as np
import ml_dtypes

N_CORES = 8
N_NODES = 512
B = N_NODES // N_CORES  # nodes per core
C = 64                  # channels
NF = 9                  # features per channel
BC = B * C              # 4096 pairs per core
G = BC // 128           # 32 partition tiles
K3, K2, K1 = 16, 4, 1
NQ = K3 + K2 + K1       # 21
ND = 4                  # output dims: idx0 d=1, idx1 d=3
MUL = 64

# Symmetric bases ------------------------------------------------------------
PAIRS = [(j, k) for j in range(NF) for k in range(j, NF)]  # 45, j<=k
TRI2 = {jk: t for t, jk in enumerate(PAIRS)}
NP2 = len(PAIRS)  # 45
SEG_OFF = []
SEG_LEN = []
_off = 0
for i in range(NF):
    SEG_OFF.append(_off)
    SEG_LEN.append(NP2 - TRI2[(i, i)])
    _off += SEG_LEN[-1]
NP3 = _off  # 165
NFEAT_TOT = NF + NP2 + NP3  # 219
CH0_N = 128
CH1_N = NFEAT_TOT - CH0_N   # 91

F_COL_X = 0
F_COL_P2 = NF          # 9
F_COL_P3 = NF + NP2    # 54

BF16 = ml_dtypes.bfloat16

N_WARM = int(os.environ.get("K_WARM", "10"))
# triple segments computed on gpsimd (rest on DVE)
GPSEGS = set(int(s) for s in os.environ.get("K_GPSEGS", "4,5,6,7,8").split(",")
             if s != "")
# W-mult engine: "pool" (Act evacs T to SBUF first) or "dve" (direct PSUM)
WMULT = os.environ.get("K_WMULT", "pool")
# f1 transpose path: "xbar" (DMA) or "pe"
F1T = os.environ.get("K_F1T", "xbar")
TB = 4   # g-tiles per T-psum batch (one PSUM bank)
EB = 8   # g-tiles per f0-transpose psum bank (bf16)

_CACHE = {}


def _mult3(i, j, k):
    if i == j == k:
        return 1.0
    if i == j or j == k or i == k:
        return 3.0
    return 6.0


def _host_pack(node_feats, node_specie,
               U3_0, U2_0, U1_0, w3_0, w2_0, w1_0,
               U3_1, U2_1, U1_1, w3_1, w2_1, w1_1,
               Wlin0, Wlin1):
    node_feats = np.asarray(node_feats, np.float32)
    spec = np.asarray(node_specie).astype(np.int64)

    # --- Ucat [219, 84] ---
    ucat = np.zeros((NFEAT_TOT, ND * NQ), np.float32)
    Us = [(np.asarray(U3_0, np.float32), np.asarray(U2_0, np.float32),
           np.asarray(U1_0, np.float32)),
          (np.asarray(U3_1, np.float32), np.asarray(U2_1, np.float32),
           np.asarray(U1_1, np.float32))]
    for D in range(ND):
        idx = 0 if D == 0 else 1
        d = 0 if D == 0 else D - 1
        U3, U2, U1 = Us[idx]
        col = D * NQ
        ucat[F_COL_X:F_COL_X + NF, col + K3 + K2] = U1[d, :, 0]
        for t, (j, k) in enumerate(PAIRS):
            m2 = 1.0 if j == k else 2.0
            ucat[F_COL_P2 + t, col + K3:col + K3 + K2] = m2 * U2[d, j, k, :]
        for i in range(NF):
            for s, (j, k) in enumerate(PAIRS[TRI2[(i, i)]:]):
                r = F_COL_P3 + SEG_OFF[i] + s
                ucat[r, col:col + K3] = _mult3(i, j, k) * U3[d, i, j, k, :]
    u0 = ucat[0:CH0_N].copy()
    u1 = ucat[CH0_N:NFEAT_TOT].copy()

    # --- per-node species weights ---
    wcat = np.concatenate([
        np.asarray(w3_0, np.float32), np.asarray(w2_0, np.float32),
        np.asarray(w1_0, np.float32), np.asarray(w3_1, np.float32),
        np.asarray(w2_1, np.float32), np.asarray(w1_1, np.float32),
    ], axis=1)                      # [NSPEC, 42, C]
    wnode = wcat[spec]              # [512, 42, C]

    # --- block-diag Wlin [2, 128, 128] (path norm 1/sqrt(C) folded in) ---
    inv_sqrt_c = 1.0 / np.sqrt(np.float32(C))
    bw = np.zeros((2, 128, 128), np.float32)
    for b2 in range(2):
        bw[0, b2 * 64:(b2 + 1) * 64, b2 * 64:(b2 + 1) * 64] = \
            np.asarray(Wlin0, np.float32) * inv_sqrt_c
        bw[1, b2 * 64:(b2 + 1) * 64, b2 * 64:(b2 + 1) * 64] = \
            np.asarray(Wlin1, np.float32) * inv_sqrt_c

    ident = np.eye(128, dtype=np.float32)

    # one [128, 552] bf16 blob: u0 | u1 | bw0 | bw1 | ident
    cblob = np.zeros((128, 552), np.float32)
    cblob[:, 0:84] = u0
    cblob[0:CH1_N, 84:168] = u1
    cblob[:, 168:296] = bw[0]
    cblob[:, 296:424] = bw[1]
    cblob[:, 424:552] = ident
    cblob = cblob.astype(BF16)

    in_maps = []
    for core in range(N_CORES):
        b0 = core * B
        # x pre-shuffled to device layout [128=(b2,c), i, g], bf16
        xs = node_feats[b0:b0 + B].reshape(G, 2, C, NF)      # [g, b2, c, i]
        xs = np.ascontiguousarray(xs.transpose(1, 2, 3, 0))  # [b2, c, i, g]
        xs = xs.reshape(128, NF, G).astype(BF16)
        wex42 = wnode[b0:b0 + B]                             # [B, 42, C]
        # natural layout [128=(b2,c), g, (D,q)=84]
        wex84 = np.concatenate(
            [wex42[:, 0:NQ]] + [wex42[:, NQ:2 * NQ]] * 3, axis=1)  # [B,84,C]
        wn = wex84.reshape(G, 2, ND * NQ, C)                 # [g, b2, 84, c]
        wn = np.ascontiguousarray(wn.transpose(1, 3, 0, 2))  # [b2, c, g, 84]
        wblob = wn.reshape(128, G, ND * NQ)
        in_maps.append({
            "x": xs,
            "cblob": cblob,
            "wblob": wblob.astype(BF16),
        })
    return in_maps


def _host_unpack(res):
    """Device returns o [128=(b2,M), 128] per core; reassemble [512, 256]."""
    out = np.zeros((N_NODES, ND * MUL), np.float32)
    for core in range(N_CORES):
        o = res[core]["o"]                       # [128, 128]
        o = o.reshape(2, MUL, 128)               # [b2, M, col]
        b0 = core * B
        # col 0..31 = g (D0);  col 32.. = (g, i)
        o0 = o[:, :, 0:G]                        # [b2, M, g]
        o1 = o[:, :, G:G + 3 * G].reshape(2, MUL, G, 3)
        for b2 in range(2):
            rows = b0 + 2 * np.arange(G) + b2    # [g]
            out[rows, 0:MUL] = o0[b2].T          # [g, M]
            cols = (MUL + 3 * np.arange(MUL)[None, :, None]
                    + np.arange(3)[None, None, :])      # [1, M, 3]
            out[rows[:, None, None], cols] = o1[b2].transpose(1, 0, 2)
    return out


def _build_nc():
    import concourse.bass as bass
    import concourse.tile as tile
    from concourse import mybir, bacc

    F32 = mybir.dt.float32
    BF = mybir.dt.bfloat16

    nc = bacc.Bacc("TRN2", target_bir_lowering=False, debug=False,
                   num_devices=N_CORES)

    x_d = nc.dram_tensor("x", [128, NF, G], BF, kind="ExternalInput").ap()
    cblob_d = nc.dram_tensor("cblob", [128, 552], BF,
                             kind="ExternalInput").ap()
    wblob_d = nc.dram_tensor("wblob", [128, G, ND * NQ], BF,
                             kind="ExternalInput").ap()
    o_d = nc.dram_tensor("o", [128, 128], F32, kind="ExternalOutput").ap()

    with tile.TileContext(nc) as tc:
        with (
            tc.tile_pool(name="sb", bufs=1) as sbp,
            tc.tile_pool(name="ps", bufs=1, space="PSUM") as psp,
        ):
            # ---- tiles (allocated once; manual rotation) ----
            f0 = sbp.tile([128, 128, G], BF)   # g-innermost (2x DVE)
            f1 = sbp.tile([128, 96, G], BF)    # rows 91..95 garbage
            cb_sb = sbp.tile([128, 552], BF)
            wb_sb = sbp.tile([128, G, ND * NQ], BF)
            ft0 = sbp.tile([128, BC], BF)
            ft1 = sbp.tile([96, BC], BF)       # rows 91..95 garbage
            tsb = [sbp.tile([128, TB, ND * NQ], BF, name=f"tsb{i}")
                   for i in range(2)]
            gsc = [sbp.tile([128, TB, ND * NQ], BF, name=f"gsc{i}")
                   for i in range(2)]
            f_sb = sbp.tile([128, G, ND], BF)
            o_sb = sbp.tile([128, 128], F32)

            warm_ps = psp.tile([128, 512], F32, name="warm")
            o_ps = psp.tile([128, 128], F32, name="ops")
            tp0 = [psp.tile([128, EB, 128], BF, name=f"tp0{i}")
                   for i in range(2)]
            tp1 = [psp.tile([96, EB, 128], BF, name=f"tp1{i}")
                   for i in range(2)]
            t_ps = [psp.tile([128, TB, ND * NQ], F32, name=f"tps{i}")
                    for i in range(2)]

            u0_sb = cb_sb[:, 0:84]
            u1_sb = cb_sb[0:CH1_N, 84:168]
            bw0_sb = cb_sb[:, 168:296]
            bw1_sb = cb_sb[:, 296:424]
            id_sb = cb_sb[:, 424:552]

            # ---- inputs: x FIRST (it gates the whole pipeline) ----
            nc.sync.dma_start(f0[:, 0:NF, :], x_d[:, 0:NF])
            nc.scalar.dma_start(cb_sb[:], cblob_d)
            half = G // 2
            nc.sync.dma_start(wb_sb[:, 0:half], wblob_d[:, 0:half])
            nc.scalar.dma_start(wb_sb[:, half:G], wblob_d[:, half:G])

            # PE warmup gated on x: flips the HAM window before the real
            # matmuls start.
            if N_WARM:
                wrhs = f0[:, 0:NF, :]
                for w in range(N_WARM):
                    nc.tensor.matmul(warm_ps[:, 0:G * NF], id_sb, wrhs,
                                     start=True, stop=True)

            # ---- monomials ----
            # pairs: rows 9..53 of F (DVE)
            for j in range(NF):
                n = NF - j
                t0 = TRI2[(j, j)]
                nc.vector.tensor_mul(
                    f0[:, F_COL_P2 + t0:F_COL_P2 + t0 + n, :],
                    f0[:, j:j + 1, :].broadcast_to([128, n, G]),
                    f0[:, j:NF, :])

            # triples: seg i = x_i * pairs[t0(i,i):], split across f0/f1
            def fseg(col, n):
                parts = []
                if col < 128:
                    w = min(n, 128 - col)
                    parts.append((f0, col, w))
                    col += w
                    n -= w
                if n > 0:
                    parts.append((f1, col - 128, n))
                return parts

            for i in range(NF):
                t0 = TRI2[(i, i)]
                o = F_COL_P3 + SEG_OFF[i]
                eng = nc.gpsimd if i in GPSEGS else nc.vector
                for tile_, c0, w in fseg(o, SEG_LEN[i]):
                    toff = F_COL_P2 + t0 + (
                        c0 + (0 if tile_ is f0 else 128) - o)
                    eng.tensor_mul(
                        tile_[:, c0:c0 + w, :],
                        f0[:, i:i + 1, :].broadcast_to([128, w, G]),
                        f0[:, toff:toff + w, :])

            # ---- transposes (PE, bf16 PSUM) + evacs ----
            # f0 transposes can start during phase B (f0 completes before
            # f1); their evacs go to Act (idle then) for groups 0-1 and DVE
            # for groups 2-3.  f1 evacs all on DVE (2x packed from bf16
            # PSUM); T-evacs (phase C) on Act.
            NGRP = G // EB
            for k in range(NGRP):
                p0 = tp0[k % 2]
                for e in range(EB):
                    g = k * EB + e
                    nc.tensor.transpose(p0[:, e], f0[:, :, g], id_sb)
                cols = slice(k * EB * 128, (k + 1) * EB * 128)
                if k < 2:
                    nc.scalar.copy(ft0[:, cols], p0[:])
                else:
                    nc.vector.tensor_scalar_mul(ft0[:, cols], p0[:], 1.0)
            for k in range(NGRP):
                p1 = tp1[k % 2]
                for e in range(EB):
                    g = k * EB + e
                    nc.tensor.transpose(p1[:, e], f1[:, :, g], id_sb)
                cols = slice(k * EB * 128, (k + 1) * EB * 128)
                nc.vector.tensor_scalar_mul(ft1[:, cols], p1[:], 1.0)

            # ---- T matmuls + species weights + q-reduce ----
            for nb in range(G // TB):
                tp = t_ps[nb % 2]
                for e in range(TB):
                    g = nb * TB + e
                    cols = slice(g * 128, (g + 1) * 128)
                    nc.tensor.matmul(tp[:, e], ft0[:, cols], u0_sb,
                                     start=True, stop=False)
                    nc.tensor.matmul(tp[:, e], ft1[0:CH1_N, cols], u1_sb,
                                     start=False, stop=True)
                gs = slice(nb * TB, (nb + 1) * TB)
                if WMULT == "pool":
                    ts = tsb[nb % 2]
                    nc.scalar.copy(ts[:], tp[:])
                    nc.gpsimd.tensor_mul(gsc[nb % 2][:], wb_sb[:, gs], ts[:])
                else:
                    nc.vector.tensor_mul(gsc[nb % 2][:], wb_sb[:, gs], tp[:])
                with nc.allow_low_precision(
                        reason="DVE reduce accumulates fp32 internally"):
                    nc.vector.tensor_reduce(
                        f_sb[:, gs], gsc[nb % 2][:].rearrange(
                            "p g (d q) -> p g d q", q=NQ),
                        axis=mybir.AxisListType.X, op=mybir.AluOpType.add)

            # ---- final linear (block-diag Wlin over channels) ----
            nc.tensor.matmul(o_ps[:, 0:G], bw0_sb, f_sb[:, :, 0],
                             start=True, stop=True)
            nc.tensor.matmul(
                o_ps[:, G:G + G * 3].rearrange("p (g i) -> p g i", g=G),
                bw1_sb, f_sb[:, :, 1:4], start=True, stop=True)

            # ---- output ----
            nc.scalar.copy(o_sb[:], o_ps[:])
            nc.sync.dma_start(o_d, o_sb[:])

    nc.compile()
    return nc


def _get_nc():
    if "nc" not in _CACHE:
        _CACHE["nc"] = _build_nc()
    return _CACHE["nc"]


def kernel(node_feats, node_specie,
           U3_0, U2_0, U1_0, w3_0, w2_0, w1_0,
           U3_1, U2_1, U1_1, w3_1, w2_1, w1_1,
           Wlin0, Wlin1):
    from concourse.bass_utils import run_bass_kernel_spmd

    in_maps = _host_pack(node_feats, node_specie,
                         U3_0, U2_0, U1_0, w3_0, w2_0, w1_0,
                         U3_1, U2_1, U1_1, w3_1, w2_1, w1_1,
                         Wlin0, Wlin1)
    nc = _get_nc()
    res = run_bass_kernel_spmd(nc, in_maps, core_ids=list(range(N_CORES)))
    return _host_unpack(res.results).astype(np.float32)


# revision 9
# speedup vs baseline: 1.5890x; 1.1112x over previous
"""Trainium2 Bass kernel for nn_EquivariantProductBasisBlock (MACE symmetric
contraction, correlation 3, irreps 0e+1o -> 0e+1o, + e3nn linear).

Strategy (data-parallel over nodes, 8 cores):
  Per core: 64 nodes x 64 channels = 4096 (b,c) pairs, each with a 9-dim
  feature vector x.  The full contraction reduces to, per pair:
      T[(D,q)] = sum_f  F[f] * Ucat[f, (D,q)]          (matmul, f = 219)
      f[D]     = sum_q  Wexp[(D,q)] * T[(D,q)]          (species weights)
      out      = blockdiag(Wlin) applied over channels  (matmul)
  where F = [x (9) | sym pairs x_j x_k (45) | sym triples x_i x_j x_k (165)]
  and Ucat folds the (symmetric) U3/U2/U1 CG tensors with permutation
  multiplicities.  Species gather + weight packing happen host-side.

Engine schedule (v2):
  DMA:   x first, then cblob/wblob; f1-chunk transposes via DMA xbar
         (SBUF->SBUF, 16x128 tiles) straight into ft1 -- no PSUM, no evac.
  PE:    warmup (HAM ramp), f0-chunk transposes (is_transpose bf16 PSUM),
         T matmuls (K=128 chunk0 + K=91 chunk1 accumulate), final linear.
  DVE:   monomial pairs + big triple segs, f0-evac (2x packed), q-reduce.
  Pool:  small triple segs, W-mult (SBUF bf16).
  Act:   T-evac (fp32 PSUM -> bf16 SBUF), output copy.
"""

import os
import sys

for _p in ("/opt/trn_rl_repo",):
    if _p not in sys.path:
        sys.path.insert(0, _p)

import numpy as np
import ml_dtypes

N_CORES = 8
N_NODES = 512
B = N_NODES // N_CORES  # nodes per core
C = 64                  # channels
NF = 9                  # features per channel
BC = B * C              # 4096 pairs per core
G = BC // 128           # 32 partition tiles
K3, K2, K1 = 16, 4, 1
NQ = K3 + K2 + K1       # 21
ND = 4                  # output dims: idx0 d=1, idx1 d=3
MUL = 64

# Symmetric bases ------------------------------------------------------------
PAIRS = [(j, k) for j in range(NF) for k in range(j, NF)]  # 45, j<=k
TRI2 = {jk: t for t, jk in enumerate(PAIRS)}
NP2 = len(PAIRS)  # 45
SEG_OFF = []
SEG_LEN = []
_off = 0
for i in range(NF):
    SEG_OFF.append(_off)
    SEG_LEN.append(NP2 - TRI2[(i, i)])
    _off += SEG_LEN[-1]
NP3 = _off  # 165
NFEAT_TOT = NF + NP2 + NP3  # 219
CH0_N = 128
CH1_N = NFEAT_TOT - CH0_N   # 91

F_COL_X = 0
F_COL_P2 = NF          # 9
F_COL_P3 = NF + NP2    # 54

BF16 = ml_dtypes.bfloat16

N_WARM = int(os.environ.get("K_WARM", "10"))
# triple segments computed on gpsimd (rest on DVE)
GPSEGS = set(int(s) for s in os.environ.get("K_GPSEGS", "5,6,7,8").split(",")
             if s != "")
# W-mult engine: "pool" (Act evacs T to SBUF first) or "dve" (direct PSUM)
WMULT = os.environ.get("K_WMULT", "pool")
# f1 transpose path: "xbar" (DMA) or "pe"
F1T = os.environ.get("K_F1T", "xbar")
TB = 4   # g-tiles per T-psum batch (one PSUM bank)
EB = 8   # g-tiles per f0-transpose psum bank (bf16)

_CACHE = {}


def _mult3(i, j, k):
    if i == j == k:
        return 1.0
    if i == j or j == k or i == k:
        return 3.0
    return 6.0


def _host_pack(node_feats, node_specie,
               U3_0, U2_0, U1_0, w3_0, w2_0, w1_0,
               U3_1, U2_1, U1_1, w3_1, w2_1, w1_1,
               Wlin0, Wlin1):
    node_feats = np.asarray(node_feats, np.float32)
    spec = np.asarray(node_specie).astype(np.int64)

    # --- Ucat [219, 84] ---
    ucat = np.zeros((NFEAT_TOT, ND * NQ), np.float32)
    Us = [(np.asarray(U3_0, np.float32), np.asarray(U2_0, np.float32),
           np.asarray(U1_0, np.float32)),
          (np.asarray(U3_1, np.float32), np.asarray(U2_1, np.float32),
           np.asarray(U1_1, np.float32))]
    for D in range(ND):
        idx = 0 if D == 0 else 1
        d = 0 if D == 0 else D - 1
        U3, U2, U1 = Us[idx]
        col = D * NQ
        ucat[F_COL_X:F_COL_X + NF, col + K3 + K2] = U1[d, :, 0]
        for t, (j, k) in enumerate(PAIRS):
            m2 = 1.0 if j == k else 2.0
            ucat[F_COL_P2 + t, col + K3:col + K3 + K2] = m2 * U2[d, j, k, :]
        for i in range(NF):
            for s, (j, k) in enumerate(PAIRS[TRI2[(i, i)]:]):
                r = F_COL_P3 + SEG_OFF[i] + s
                ucat[r, col:col + K3] = _mult3(i, j, k) * U3[d, i, j, k, :]
    u0 = ucat[0:CH0_N].copy()
    u1 = ucat[CH0_N:NFEAT_TOT].copy()

    # --- per-node species weights ---
    wcat = np.concatenate([
        np.asarray(w3_0, np.float32), np.asarray(w2_0, np.float32),
        np.asarray(w1_0, np.float32), np.asarray(w3_1, np.float32),
        np.asarray(w2_1, np.float32), np.asarray(w1_1, np.float32),
    ], axis=1)                      # [NSPEC, 42, C]
    wnode = wcat[spec]              # [512, 42, C]

    # --- block-diag Wlin [2, 128, 128] (path norm 1/sqrt(C) folded in) ---
    inv_sqrt_c = 1.0 / np.sqrt(np.float32(C))
    bw = np.zeros((2, 128, 128), np.float32)
    for b2 in range(2):
        bw[0, b2 * 64:(b2 + 1) * 64, b2 * 64:(b2 + 1) * 64] = \
            np.asarray(Wlin0, np.float32) * inv_sqrt_c
        bw[1, b2 * 64:(b2 + 1) * 64, b2 * 64:(b2 + 1) * 64] = \
            np.asarray(Wlin1, np.float32) * inv_sqrt_c

    ident = np.eye(128, dtype=np.float32)

    # one [128, 552] bf16 blob: u0 | u1 | bw0 | bw1 | ident
    cblob = np.zeros((128, 552), np.float32)
    cblob[:, 0:84] = u0
    cblob[0:CH1_N, 84:168] = u1
    cblob[:, 168:296] = bw[0]
    cblob[:, 296:424] = bw[1]
    cblob[:, 424:552] = ident
    cblob = cblob.astype(BF16)

    in_maps = []
    for core in range(N_CORES):
        b0 = core * B
        # x pre-shuffled to device layout [128=(b2,c), i, g], bf16
        xs = node_feats[b0:b0 + B].reshape(G, 2, C, NF)      # [g, b2, c, i]
        xs = np.ascontiguousarray(xs.transpose(1, 2, 3, 0))  # [b2, c, i, g]
        xs = xs.reshape(128, NF, G).astype(BF16)
        wex42 = wnode[b0:b0 + B]                             # [B, 42, C]
        # natural layout [128=(b2,c), g, (D,q)=84]
        wex84 = np.concatenate(
            [wex42[:, 0:NQ]] + [wex42[:, NQ:2 * NQ]] * 3, axis=1)  # [B,84,C]
        wn = wex84.reshape(G, 2, ND * NQ, C)                 # [g, b2, 84, c]
        wn = np.ascontiguousarray(wn.transpose(1, 3, 0, 2))  # [b2, c, g, 84]
        wblob = wn.reshape(128, G, ND * NQ)
        in_maps.append({
            "x": xs,
            "cblob": cblob,
            "wblob": wblob.astype(BF16),
        })
    return in_maps


def _host_unpack(res):
    """Device returns o [128=(b2,M), 128] per core; reassemble [512, 256]."""
    out = np.zeros((N_NODES, ND * MUL), np.float32)
    for core in range(N_CORES):
        o = res[core]["o"]                       # [128, 128]
        o = o.reshape(2, MUL, 128)               # [b2, M, col]
        b0 = core * B
        # col 0..31 = g (D0);  col 32.. = (g, i)
        o0 = o[:, :, 0:G]                        # [b2, M, g]
        o1 = o[:, :, G:G + 3 * G].reshape(2, MUL, G, 3)
        for b2 in range(2):
            rows = b0 + 2 * np.arange(G) + b2    # [g]
            out[rows, 0:MUL] = o0[b2].T          # [g, M]
            cols = (MUL + 3 * np.arange(MUL)[None, :, None]
                    + np.arange(3)[None, None, :])      # [1, M, 3]
            out[rows[:, None, None], cols] = o1[b2].transpose(1, 0, 2)
    return out


def _build_nc():
    import concourse.bass as bass
    import concourse.tile as tile
    from concourse import mybir, bacc

    F32 = mybir.dt.float32
    BF = mybir.dt.bfloat16

    nc = bacc.Bacc("TRN2", target_bir_lowering=False, debug=False,
                   num_devices=N_CORES)

    x_d = nc.dram_tensor("x", [128, NF, G], BF, kind="ExternalInput").ap()
    cblob_d = nc.dram_tensor("cblob", [128, 552], BF,
                             kind="ExternalInput").ap()
    wblob_d = nc.dram_tensor("wblob", [128, G, ND * NQ], BF,
                             kind="ExternalInput").ap()
    o_d = nc.dram_tensor("o", [128, 128], F32, kind="ExternalOutput").ap()

    with tile.TileContext(nc) as tc:
        with (
            tc.tile_pool(name="sb", bufs=1) as sbp,
            tc.tile_pool(name="ps", bufs=1, space="PSUM") as psp,
        ):
            # ---- tiles (allocated once; manual rotation) ----
            f0 = sbp.tile([128, 128, G], BF)   # g-innermost (2x DVE)
            f1 = sbp.tile([128, 128, G], BF)   # rows 91..127 garbage (full
            #   128 rows so every PE transpose has M=128 -- M<128 runs 3x
            #   slower on hardware)
            cb_sb = sbp.tile([128, 552], BF)
            wb_sb = sbp.tile([128, G, ND * NQ], BF)
            ft0 = sbp.tile([128, BC], BF)
            ft1 = sbp.tile([128, BC], BF)      # rows 91.. garbage
            tsb = [sbp.tile([128, TB, ND * NQ], BF, name=f"tsb{i}")
                   for i in range(2)]
            gsc = [sbp.tile([128, TB, ND * NQ], BF, name=f"gsc{i}")
                   for i in range(2)]
            f_sb = sbp.tile([128, G, ND], BF)
            o_sb = sbp.tile([128, 128], F32)

            warm_ps = psp.tile([128, 512], F32, name="warm")
            o_ps = psp.tile([128, 128], F32, name="ops")
            tp0 = [psp.tile([128, EB, 128], BF, name=f"tp0{i}")
                   for i in range(2)]
            tp1 = [psp.tile([128, EB, 128], BF, name=f"tp1{i}")
                   for i in range(2)]
            t_ps = [psp.tile([128, TB, ND * NQ], F32, name=f"tps{i}")
                    for i in range(2)]

            u0_sb = cb_sb[:, 0:84]
            u1_sb = cb_sb[0:CH1_N, 84:168]
            bw0_sb = cb_sb[:, 168:296]
            bw1_sb = cb_sb[:, 296:424]
            id_sb = cb_sb[:, 424:552]

            # ---- inputs: x FIRST (it gates the whole pipeline) ----
            nc.sync.dma_start(f0[:, 0:NF, :], x_d[:, 0:NF])
            nc.scalar.dma_start(cb_sb[:], cblob_d)
            half = G // 2
            nc.sync.dma_start(wb_sb[:, 0:half], wblob_d[:, 0:half])
            nc.scalar.dma_start(wb_sb[:, half:G], wblob_d[:, half:G])

            # PE warmup gated on x: flips the HAM window before the real
            # matmuls start.
            if N_WARM:
                wrhs = f0[:, 0:NF, :]
                for w in range(N_WARM):
                    nc.tensor.matmul(warm_ps[:, 0:G * NF], id_sb, wrhs,
                                     start=True, stop=True)

            # ---- monomials ----
            # pairs: rows 9..53 of F (DVE)
            for j in range(NF):
                n = NF - j
                t0 = TRI2[(j, j)]
                nc.vector.tensor_mul(
                    f0[:, F_COL_P2 + t0:F_COL_P2 + t0 + n, :],
                    f0[:, j:j + 1, :].broadcast_to([128, n, G]),
                    f0[:, j:NF, :])

            # triples: seg i = x_i * pairs[t0(i,i):], split across f0/f1
            def fseg(col, n):
                parts = []
                if col < 128:
                    w = min(n, 128 - col)
                    parts.append((f0, col, w))
                    col += w
                    n -= w
                if n > 0:
                    parts.append((f1, col - 128, n))
                return parts

            for i in range(NF):
                t0 = TRI2[(i, i)]
                o = F_COL_P3 + SEG_OFF[i]
                eng = nc.gpsimd if i in GPSEGS else nc.vector
                for tile_, c0, w in fseg(o, SEG_LEN[i]):
                    toff = F_COL_P2 + t0 + (
                        c0 + (0 if tile_ is f0 else 128) - o)
                    eng.tensor_mul(
                        tile_[:, c0:c0 + w, :],
                        f0[:, i:i + 1, :].broadcast_to([128, w, G]),
                        f0[:, toff:toff + w, :])

            # ---- transposes (PE, bf16 PSUM) + evacs + T-matmul batches ----
            # f0 transposes start during phase B (f0 completes before f1);
            # their evacs go to Act (idle then) for groups 0-1 and DVE for
            # groups 2-3.  f1 transposes interleave with the T batches on
            # the PE stream; f1 evacs mostly on Act, T-evacs on Act, W-mult
            # on Pool, q-reduce on DVE.
            NGRP = G // EB

            def f0_group(k):
                p0 = tp0[k % 2]
                for e in range(EB):
                    nc.tensor.transpose(p0[:, e], f0[:, :, k * EB + e],
                                        id_sb)
                cols = slice(k * EB * 128, (k + 1) * EB * 128)
                if k < 2:
                    nc.scalar.copy(ft0[:, cols], p0[:])
                else:
                    nc.vector.tensor_scalar_mul(ft0[:, cols], p0[:], 1.0)

            def f1_group(k):
                p1 = tp1[k % 2]
                for e in range(EB):
                    nc.tensor.transpose(p1[:, e], f1[:, :, k * EB + e],
                                        id_sb)
                cols = slice(k * EB * 128, (k + 1) * EB * 128)
                if k < 3:
                    nc.scalar.copy(ft1[:, cols], p1[:])
                else:
                    nc.vector.tensor_scalar_mul(ft1[:, cols], p1[:], 1.0)

            def t_batch(nb):
                tp = t_ps[nb % 2]
                for e in range(TB):
                    g = nb * TB + e
                    cols = slice(g * 128, (g + 1) * 128)
                    nc.tensor.matmul(tp[:, e], ft0[:, cols], u0_sb,
                                     start=True, stop=False)
                    nc.tensor.matmul(tp[:, e], ft1[0:CH1_N, cols], u1_sb,
                                     start=False, stop=True)
                gs = slice(nb * TB, (nb + 1) * TB)
                if WMULT == "pool":
                    ts = tsb[nb % 2]
                    nc.scalar.copy(ts[:], tp[:])
                    nc.gpsimd.tensor_mul(gsc[nb % 2][:], wb_sb[:, gs], ts[:])
                else:
                    nc.vector.tensor_mul(gsc[nb % 2][:], wb_sb[:, gs], tp[:])
                with nc.allow_low_precision(
                        reason="DVE reduce accumulates fp32 internally"):
                    nc.vector.tensor_reduce(
                        f_sb[:, gs], gsc[nb % 2][:].rearrange(
                            "p g (d q) -> p g d q", q=NQ),
                        axis=mybir.AxisListType.X, op=mybir.AluOpType.add)

            for k in range(NGRP):
                f0_group(k)
            f1_group(0)
            f1_group(1)
            t_batch(0)
            t_batch(1)
            f1_group(2)
            t_batch(2)
            t_batch(3)
            f1_group(3)
            for nb in range(4, G // TB):
                t_batch(nb)

            # ---- final linear (block-diag Wlin over channels) ----
            nc.tensor.matmul(o_ps[:, 0:G], bw0_sb, f_sb[:, :, 0],
                             start=True, stop=True)
            nc.tensor.matmul(
                o_ps[:, G:G + G * 3].rearrange("p (g i) -> p g i", g=G),
                bw1_sb, f_sb[:, :, 1:4], start=True, stop=True)

            # ---- output ----
            nc.scalar.copy(o_sb[:], o_ps[:])
            nc.sync.dma_start(o_d, o_sb[:])

    nc.compile()
    return nc


def _get_nc():
    if "nc" not in _CACHE:
        _CACHE["nc"] = _build_nc()
    return _CACHE["nc"]


def kernel(node_feats, node_specie,
           U3_0, U2_0, U1_0, w3_0, w2_0, w1_0,
           U3_1, U2_1, U1_1, w3_1, w2_1, w1_1,
           Wlin0, Wlin1):
    from concourse.bass_utils import run_bass_kernel_spmd

    in_maps = _host_pack(node_feats, node_specie,
                         U3_0, U2_0, U1_0, w3_0, w2_0, w1_0,
                         U3_1, U2_1, U1_1, w3_1, w2_1, w1_1,
                         Wlin0, Wlin1)
    nc = _get_nc()
    res = run_bass_kernel_spmd(nc, in_maps, core_ids=list(range(N_CORES)))
    return _host_unpack(res.results).astype(np.float32)
